# revision 2
# baseline (speedup 1.0000x reference)
"""AutoCorrelation block fully on-device (8 NeuronCores).

Core c = 2b+g (b batch, g head-group of 8 heads). Pipeline per core:
  P1  PE-transpose q,k halves (f32)         -> qhT, khT  (DRAM)
  P2  projections Q^T,K^T (f32, +bias), V^T (bf16, +bias)
  P3  pair AllGather -> full-L Q^T,K^T,V^T for the batch
  P5  per head h: M = Q K^T strips (f32 PE), skew-written to DRAM so the
      circular-diagonal sums become plain strided reads; partition-reduce
      -> exact f32 corr_mean; top-3 via max_with_indices; softmax weights;
      V rolled by each delay via runtime-offset DMA from [V^T|V^T]; weighted
      sum -> accV (d-partitioned, bf16)
  P6  out = sum_h accV_h^T @ WoT_h  (bf16 matmul, f32 psum)
  P7  pair ReduceScatter of the partial output
  P8  + bo, cast bf16, write y (2048, 1024)

Numerics: q,k,Wq,Wk f32 end-to-end for delay selection (corr err ~1e-4 vs
min top3-gap 8.4e-3). v/Wv/Wo path bf16.
"""
import os
import sys

import numpy as np

try:
    import concourse.bass  # noqa: F401
except ImportError:
    sys.path.insert(0, "/opt/trn_rl_repo")

B, L, DM = 4, 4096, 1024
H, D, TOPK = 16, 64, 3
NCORES = 8
RH = 2048           # rows per core (half batch)
P = 4224            # Mt row pitch (f32): P*4 % 256 == 0 not required now, P-1 >= 4223
GROUPS2 = [[0, 1], [2, 3], [4, 5], [6, 7]]
GROUPS8 = [[0, 1, 2, 3, 4, 5, 6, 7]]
WF_ROWS = 2056      # f32 blob: WqT(1024) WkT(1024) bq bk bo bv pad
WB_ROWS = 2056      # bf16 blob: WvT(1024) WoT(1024) pad
WSL = WF_ROWS // NCORES  # 257
PAIR_COLL = bool(int(os.environ.get("KV2_PAIR_COLL", "1")))

_NC = None
LAST_EXEC_NS = None
LAST_RUN_S = None


def _build_nc():
    import concourse.bass as bass
    import concourse.mybir as mybir
    import concourse.tile as tile
    from concourse import bacc
    from concourse.ap import AP

    F32, BF16 = mybir.dt.float32, mybir.dt.bfloat16
    U32 = mybir.dt.uint32
    ALU = mybir.AluOpType
    ACT = mybir.ActivationFunctionType
    PS = bass.MemorySpace.PSUM

    nc = bacc.Bacc(None, target_bir_lowering=False, num_devices=NCORES)

    qh = nc.dram_tensor("qh", (RH, DM), F32, kind="ExternalInput")
    kh = nc.dram_tensor("kh", (RH, DM), F32, kind="ExternalInput")
    vh = nc.dram_tensor("vh", (RH, DM), BF16, kind="ExternalInput")
    wf_sl = nc.dram_tensor("wf_sl", (WSL, DM), F32, kind="ExternalInput")
    wb_sl = nc.dram_tensor("wb_sl", (WSL, DM), BF16, kind="ExternalInput")
    ident_in = nc.dram_tensor("ident", (128, 128), F32, kind="ExternalInput")
    gsel = nc.dram_tensor("gsel", (1, 2), F32, kind="ExternalInput")  # one-hot of g
    y = nc.dram_tensor("y", (RH, DM), BF16, kind="ExternalOutput")
    DBG = bool(int(os.environ.get("KV2_DEBUG", "0")))
    if DBG:
        corr_dbg = nc.dram_tensor("corr_dbg", (8, L), F32, kind="ExternalOutput")
        ci_dbg = nc.dram_tensor("ci_dbg", (8, 8), U32, kind="ExternalOutput")
        w3_dbg = nc.dram_tensor("w3_dbg", (8, 8), F32, kind="ExternalOutput")
        qt_dbg = nc.dram_tensor("qt_dbg", (DM, RH), F32, kind="ExternalOutput")
        vt_dbg = nc.dram_tensor("vt_dbg", (RH, DM), BF16, kind="ExternalOutput")
        av_dbg = nc.dram_tensor("av_dbg", (8, D, L), BF16, kind="ExternalOutput")

    NAG = 2 if PAIR_COLL else NCORES   # replicas seen in projection AllGather

    with tile.TileContext(nc) as tc:
        cpool = tc.alloc_tile_pool(name="const", bufs=1)
        drp = tc.alloc_tile_pool(name="dram", bufs=1, space="DRAM")

        ident = cpool.tile([128, 128], F32, name="identt")
        nc.sync.dma_start(ident[:], ident_in[:])
        ones = cpool.tile([128, 1], F32, name="ones")
        nc.vector.memset(ones[:], 1.0)
        gsel_sb = cpool.tile([1, 2], F32, name="gsel_sb")
        nc.sync.dma_start(gsel_sb[:], gsel[:])
        g0b = cpool.tile([128, 1], F32, name="g0b")
        g1b = cpool.tile([128, 1], F32, name="g1b")
        nc.gpsimd.partition_broadcast(g0b[:], gsel_sb[0:1, 0:1])
        nc.gpsimd.partition_broadcast(g1b[:], gsel_sb[0:1, 1:2])

        # ---------- weight AllGather (8-core) ----------
        WF = drp.tile([WF_ROWS, DM], F32, name="WF")
        WB = drp.tile([WB_ROWS, DM], BF16, name="WB")
        wfb = drp.tile([WSL, DM], F32, name="wfb")
        wbb = drp.tile([WSL, DM], BF16, name="wbb")
        nc.sync.dma_start(wfb[:], wf_sl[:])
        nc.sync.dma_start(wbb[:], wb_sl[:])
        nc.gpsimd.collective_compute("AllGather", ALU.bypass, replica_groups=GROUPS8,
                                     ins=[wfb.opt()], outs=[WF.opt()])
        nc.gpsimd.collective_compute("AllGather", ALU.bypass, replica_groups=GROUPS8,
                                     ins=[wbb.opt()], outs=[WB.opt()])

        bq_sb = cpool.tile([128, 8], F32, name="bq_sb")   # [o%128, o//128]
        bk_sb = cpool.tile([128, 8], F32, name="bk_sb")
        for t_sb, row in ((bq_sb, 2048), (bk_sb, 2049)):
            src = AP(tensor=WF[:].tensor, offset=row * DM, ap=[[1, 128], [128, 8]])
            nc.sync.dma_start(t_sb[:], src)
        # row index base for the V-roll gathers: iof[p, j] = 128j + p (f32)
        io32 = cpool.tile([128, 32], mybir.dt.int32, name="io32")
        nc.gpsimd.iota(io32[:], pattern=[[128, 32]], base=0, channel_multiplier=1)
        iof = cpool.tile([128, 32], F32, name="iof")
        nc.vector.tensor_copy(iof[:], io32[:])

        # ---------- DRAM intermediates ----------
        qhT = drp.tile([DM, RH], F32, name="qhT")
        khT = drp.tile([DM, RH], F32, name="khT")
        QTh = drp.tile([DM, RH], F32, name="QTh")
        KTh = drp.tile([DM, RH], F32, name="KTh")
        Vrows = drp.tile([RH, DM], BF16, name="Vrows")
        AGQ = drp.tile([NAG, DM, RH], F32, name="AGQ")
        AGK = drp.tile([NAG, DM, RH], F32, name="AGK")
        AGV = drp.tile([NAG, RH, DM], BF16, name="AGV")
        Vp1 = drp.tile([L, D], BF16, name="Vp1")
        Mt = drp.tile([L, P], F32, name="Mt")
        accV_d = drp.tile([8, D, L], BF16, name="accV_d")
        opart = drp.tile([L, DM], F32, name="opart")
        rsout = drp.tile([RH, DM], F32, name="rsout")

        # ---------- P1: transpose qh, kh (f32, PE) ----------
        with tc.tile_pool(name="p1", bufs=2) as wp, \
             tc.tile_pool(name="p1ps", bufs=4, space=PS) as pp:
            for src_, dst in ((qh, qhT), (kh, khT)):
                for tt in range(16):
                    xt = wp.tile([128, DM], F32, tag="xt")
                    nc.sync.dma_start(xt[:], src_[tt * 128:(tt + 1) * 128, :])
                    for ic in range(8):
                        pt = pp.tile([128, 128], F32, tag="pt")
                        nc.tensor.transpose(pt[:], xt[:, ic * 128:(ic + 1) * 128],
                                            ident[:])
                        st = wp.tile([128, 128], F32, tag="st")
                        nc.vector.tensor_copy(st[:], pt[:])
                        nc.sync.dma_start(
                            dst[ic * 128:(ic + 1) * 128, tt * 128:(tt + 1) * 128],
                            st[:])

        # ---------- P2: projections (all transposed orientation) ----------
        with tc.tile_pool(name="p2w", bufs=1) as wgt, \
             tc.tile_pool(name="p2", bufs=2) as wp, \
             tc.tile_pool(name="p2ps", bufs=4, space=PS) as pp:
            def ld_wT(rows0, blob, dt, name):
                t = wgt.tile([128, 8, DM], dt, name=name)
                src = AP(tensor=blob[:].tensor, offset=rows0 * DM,
                         ap=[[DM, 128], [128 * DM, 8], [1, DM]])
                nc.sync.dma_start(t[:], src)
                return t
            WqT_sb = ld_wT(0, WF, F32, "WqT_sb")
            WkT_sb = ld_wT(1024, WF, F32, "WkT_sb")
            WvT_sb = ld_wT(0, WB, BF16, "WvT_sb")
            vhT = wgt.tile([128, 8, RH], BF16, name="vhT")
            for ic in range(8):
                nc.sync.dma_start_transpose(vhT[:, ic, :],
                                            vh[:, ic * 128:(ic + 1) * 128])

            # f32 Q^T/K^T: lhsT = W chunk, rhs = xT slab (f32)
            for xT, wsb, bsb, dstT in ((qhT, WqT_sb, bq_sb, QTh),
                                       (khT, WkT_sb, bk_sb, KTh)):
                for ts in range(4):
                    xs = wp.tile([128, 8, 512], F32, tag="xs")
                    for ic in range(8):
                        nc.sync.dma_start(
                            xs[:, ic, :],
                            xT[ic * 128:(ic + 1) * 128, ts * 512:(ts + 1) * 512])
                    for ot in range(8):
                        ps = pp.tile([128, 512], F32, tag="ps")
                        for ic in range(8):
                            nc.tensor.matmul(
                                ps[:], wsb[:, ic, ot * 128:(ot + 1) * 128],
                                xs[:, ic, :], start=(ic == 0), stop=(ic == 7))
                        st = wp.tile([128, 512], F32, tag="st")
                        nc.vector.tensor_scalar(st[:], ps[:], bsb[:, ot:ot + 1],
                                                None, op0=ALU.add)
                        nc.sync.dma_start(
                            dstT[ot * 128:(ot + 1) * 128, ts * 512:(ts + 1) * 512],
                            st[:])

            # bf16 V rows: lhsT = vhT chunk (stationary), rhs = WvT chunk
            bv_row = wgt.tile([1, DM], F32, name="bv_row")
            nc.sync.dma_start(bv_row[:], WF[2051:2052, :])
            bvb = wgt.tile([128, DM], F32, name="bvb")
            nc.gpsimd.partition_broadcast(bvb[:], bv_row[:])
            for tt in range(16):
                for osl in range(2):
                    ps = pp.tile([128, 512], F32, tag="ps")
                    for ic in range(8):
                        nc.tensor.matmul(
                            ps[:], vhT[:, ic, tt * 128:(tt + 1) * 128],
                            WvT_sb[:, ic, osl * 512:(osl + 1) * 512],
                            start=(ic == 0), stop=(ic == 7))
                    st = wp.tile([128, 512], BF16, tag="stv")
                    nc.vector.tensor_add(st[:], ps[:],
                                         bvb[:, osl * 512:(osl + 1) * 512])
                    nc.sync.dma_start(
                        Vrows[tt * 128:(tt + 1) * 128, osl * 512:(osl + 1) * 512],
                        st[:])

        if DBG:
            nc.sync.dma_start(qt_dbg[:], QTh[:])
            nc.sync.dma_start(vt_dbg[:], Vrows[:])
        # ---------- P3: AllGather of projections ----------
        pg = GROUPS2 if PAIR_COLL else GROUPS8
        for src_, dst in ((QTh, AGQ), (KTh, AGK), (Vrows, AGV)):
            nc.gpsimd.collective_compute("AllGather", ALU.bypass, replica_groups=pg,
                                         ins=[src_.opt()], outs=[dst.opt()])

        # ---------- P5: per-head ----------
        with tc.tile_pool(name="hd", bufs=1) as hp:
            ACC = hp.tile([128, L], F32, name="ACCt")
            CORR = hp.tile([1, L], F32, name="CORRt")
            accVb = hp.tile([64, L], BF16, name="accVbt")

            for h in range(8):
                with tc.tile_pool(name="qk", bufs=1) as qk:
                    QT_sb = qk.tile([64, L], F32, tag="QT_sb")
                    KT_sb = qk.tile([64, L], F32, tag="KT_sb")
                    tmp = qk.tile([64, L], F32, tag="tmpf")

                    for dstT, ag in ((QT_sb, AGQ), (KT_sb, AGK)):
                        for r in range(2):
                            nc.sync.dma_start(
                                dstT[:, r * RH:(r + 1) * RH],
                                ag[r, 64 * h:64 * h + 64, :])
                            nc.sync.dma_start(
                                tmp[:, r * RH:(r + 1) * RH],
                                ag[r, 512 + 64 * h:512 + 64 * h + 64, :])
                        nc.vector.tensor_scalar(dstT[:], dstT[:], g0b[0:64, :],
                                                None, op0=ALU.mult)
                        nc.vector.scalar_tensor_tensor(
                            dstT[:], tmp[:], g1b[0:64, :], dstT[:],
                            op0=ALU.mult, op1=ALU.add)
                    # stage this head's V rows (g-blended) into Vp1 (L, D)
                    vs0 = qk.tile([128, 32, D], BF16, tag="vs0")
                    vs1 = qk.tile([128, 32, D], BF16, tag="vs1")
                    for r in range(2):
                        for q_, off in ((vs0, 64 * h), (vs1, 512 + 64 * h)):
                            srcv = AP(tensor=AGV[:].tensor,
                                      offset=r * RH * DM + off,
                                      ap=[[DM, 128], [128 * DM, 16], [1, D]])
                            nc.sync.dma_start(q_[:, r * 16:(r + 1) * 16, :], srcv)
                    nc.vector.tensor_scalar(vs0[:], vs0[:], g0b[:], None,
                                            op0=ALU.mult)
                    nc.vector.scalar_tensor_tensor(vs0[:], vs1[:], g1b[:], vs0[:],
                                                   op0=ALU.mult, op1=ALU.add)
                    vp_dst = AP(tensor=Vp1[:].tensor, offset=0,
                                ap=[[D, 128], [128 * D, 32], [1, D]])
                    nc.sync.dma_start(vp_dst, vs0[:])

                    # ---- M strips (f32) -> skewed Mt -> strided reads -> ACC
                    with tc.tile_pool(name="mst", bufs=2) as sp, \
                         tc.tile_pool(name="mps", bufs=2, space=PS) as mp:
                        for J in range(32):
                            strip = sp.tile([128, L], F32, tag="strip")
                            for halfp in range(2):
                                ps = mp.tile([128, 2048], F32, tag="mm")
                                for isl in range(4):
                                    s = halfp * 4 + isl
                                    nc.tensor.matmul(
                                        ps[:, isl * 512:(isl + 1) * 512],
                                        KT_sb[:, J * 128:(J + 1) * 128],
                                        QT_sb[:, s * 512:(s + 1) * 512],
                                        start=True, stop=True,
                                        skip_group_check=True)
                                nc.vector.tensor_copy(
                                    strip[:, halfp * 2048:(halfp + 1) * 2048],
                                    ps[:])
                            main = AP(tensor=Mt[:].tensor, offset=128 * J * P,
                                      ap=[[P - 1, 128], [1, L]])
                            nc.sync.dma_start(main, strip[:])
                            wrap = AP(tensor=Mt[:].tensor, offset=128 * J * P + L,
                                      ap=[[P - 1, 128], [1, 127]])
                            nc.sync.dma_start(wrap, strip[:, 0:127])
                        for J in range(32):
                            sk = sp.tile([128, L], F32, tag="sk")
                            src = AP(tensor=Mt[:].tensor, offset=128 * J * P,
                                     ap=[[P, 128], [1, L]])
                            nc.sync.dma_start(sk[:], src)
                            off = 128 * J
                            if J == 0:
                                nc.vector.tensor_copy(ACC[:], sk[:])
                            else:
                                nc.vector.tensor_add(ACC[:, 0:L - off],
                                                     ACC[:, 0:L - off],
                                                     sk[:, off:L])
                                nc.vector.tensor_add(ACC[:, L - off:L],
                                                     ACC[:, L - off:L],
                                                     sk[:, 0:off])

                    with tc.tile_pool(name="rps", bufs=2, space=PS) as rp:
                        for ns in range(8):
                            pc = rp.tile([1, 512], F32, tag="pc")
                            nc.tensor.matmul(pc[:], ones[:],
                                             ACC[:, ns * 512:(ns + 1) * 512],
                                             start=True, stop=True)
                            nc.vector.tensor_scalar(
                                CORR[0:1, ns * 512:(ns + 1) * 512], pc[:],
                                1.0 / 64, None, op0=ALU.mult)

                    # top-3 delays + softmax weights
                    cv = hp.tile([1, 8], F32, tag="cv")
                    ci = hp.tile([1, 8], U32, tag="ci")
                    nc.vector.max_with_indices(cv[:], ci[:], CORR[:])
                    ex = hp.tile([1, 8], F32, tag="ex")
                    nc.vector.tensor_scalar(ex[:], cv[:], cv[0:1, 0:1], None,
                                            op0=ALU.subtract)
                    nc.scalar.activation(ex[:], ex[:], ACT.Exp)
                    sm = hp.tile([1, 1], F32, tag="sm")
                    nc.vector.tensor_reduce(sm[:], ex[0:1, 0:TOPK],
                                            axis=mybir.AxisListType.X, op=ALU.add)
                    si = hp.tile([1, 1], F32, tag="si")
                    nc.vector.reciprocal(si[:], sm[:])
                    w3 = hp.tile([1, 8], F32, tag="w3")
                    nc.vector.tensor_scalar(w3[:], ex[:], si[0:1, 0:1], None,
                                            op0=ALU.mult)
                    w3b = hp.tile([128, 8], F32, tag="w3b")
                    nc.gpsimd.partition_broadcast(w3b[:], w3[:])

                    # rolled-V weighted sum via indirect row gathers
                    cif = hp.tile([1, 8], F32, tag="cif")
                    nc.vector.tensor_copy(cif[:], ci[:])
                    taub = hp.tile([128, 8], F32, tag="taub")
                    nc.gpsimd.partition_broadcast(taub[:], cif[:])
                    ACCW = hp.tile([128, 32, D], F32, tag="ACCW")
                    for cd in range(TOPK):
                        idxf = hp.tile([128, 32], F32, tag="idxf")
                        nc.vector.tensor_scalar(idxf[:], iof[:],
                                                taub[:, cd:cd + 1], None,
                                                op0=ALU.subtract)
                        mkt = hp.tile([128, 32], F32, tag="mkt")
                        nc.vector.tensor_scalar(mkt[:], idxf[:], 0.0, None,
                                                op0=ALU.is_lt)
                        nc.vector.scalar_tensor_tensor(idxf[:], mkt[:], float(L),
                                                       idxf[:], op0=ALU.mult,
                                                       op1=ALU.add)
                        idx32 = hp.tile([128, 32], mybir.dt.int32, tag="idx32")
                        nc.vector.tensor_copy(idx32[:], idxf[:])
                        vr = qk.tile([128, 32, D], BF16, tag="vr")
                        for J in range(32):
                            nc.gpsimd.indirect_dma_start(
                                out=vr[:, J, :], out_offset=None, in_=Vp1[:],
                                in_offset=bass.IndirectOffsetOnAxis(
                                    ap=idx32[:, J:J + 1], axis=0))
                        if cd == 0:
                            nc.vector.tensor_scalar(ACCW[:], vr[:],
                                                    w3b[:, 0:1], None,
                                                    op0=ALU.mult)
                        else:
                            nc.vector.scalar_tensor_tensor(
                                ACCW[:], vr[:], w3b[:, cd:cd + 1], ACCW[:],
                                op0=ALU.mult, op1=ALU.add)
                    # transpose to (d, t) and store
                    with tc.tile_pool(name="tps", bufs=4, space=PS) as tp2:
                        for T in range(32):
                            pt = tp2.tile([64, 128], F32, tag="pt")
                            nc.tensor.transpose(pt[:], ACCW[:, T, :], ident[:])
                            nc.vector.tensor_copy(
                                accVb[:, T * 128:(T + 1) * 128], pt[:])
                    nc.sync.dma_start(accV_d[h], accVb[:])
                    if DBG:
                        nc.sync.dma_start(corr_dbg[h:h+1, :], CORR[:])
                        nc.sync.dma_start(ci_dbg[h:h+1, :], ci[:])
                        nc.sync.dma_start(w3_dbg[h:h+1, :], w3[:])
                        nc.sync.dma_start(av_dbg[h], accVb[:])

        # ---------- P6: output projection ----------
        with tc.tile_pool(name="p6w", bufs=1) as w6, \
             tc.tile_pool(name="p6", bufs=2) as wp, \
             tc.tile_pool(name="p6ps", bufs=2, space=PS) as pp:
            WoT_all = w6.tile([64, 8, DM], BF16, name="WoT_all")
            wo1 = w6.tile([64, 8, DM], BF16, name="wo1")
            for q_, goff in ((WoT_all, 0), (wo1, 512)):
                src = AP(tensor=WB[:].tensor, offset=(1024 + goff) * DM,
                         ap=[[DM, 64], [64 * DM, 8], [1, DM]])
                nc.sync.dma_start(q_[:], src)
            nc.vector.tensor_scalar(WoT_all[:], WoT_all[:], g0b[0:64, :], None,
                                    op0=ALU.mult)
            nc.vector.scalar_tensor_tensor(WoT_all[:], wo1[:], g1b[0:64, :],
                                           WoT_all[:], op0=ALU.mult, op1=ALU.add)
            for tt in range(32):
                avs = wp.tile([64, 8, 128], BF16, tag="avs")
                for h in range(8):
                    nc.sync.dma_start(avs[:, h, :],
                                      accV_d[h, :, tt * 128:(tt + 1) * 128])
                for ns in range(2):
                    ps = pp.tile([128, 512], F32, tag="ps")
                    for h in range(8):
                        nc.tensor.matmul(ps[:], avs[:, h, :],
                                         WoT_all[:, h, ns * 512:(ns + 1) * 512],
                                         start=(h == 0), stop=(h == 7))
                    st = wp.tile([128, 512], F32, tag="st")
                    nc.vector.tensor_copy(st[:], ps[:])
                    nc.sync.dma_start(
                        opart[tt * 128:(tt + 1) * 128, ns * 512:(ns + 1) * 512],
                        st[:])

        # ---------- P7: pair partial-sum ----------
        if PAIR_COLL:
            nc.gpsimd.collective_compute("ReduceScatter", ALU.add,
                                         replica_groups=GROUPS2,
                                         ins=[opart.opt()], outs=[rsout.opt()])
        else:
            AGO = drp.tile([NCORES, RH, DM], F32, name="AGO")
            half = drp.tile([RH, DM], F32, name="half")
            # each core contributes the rows its PAIR needs? -> allgather my
            # partial's both halves is 2x; instead allgather full partials.
            AGO2 = drp.tile([NCORES, L, DM], F32, name="AGO2")
            nc.gpsimd.collective_compute("AllGather", ALU.bypass,
                                         replica_groups=GROUPS8,
                                         ins=[opart.opt()], outs=[AGO2.opt()])
            # rsout = AGO2[2b][g*RH:...] + AGO2[2b+1][g*RH:...]; rank
            # selection depends on my core id -> use partition-id? Simplest:
            # every core reduces with a gsel/core-id blend is complex; use
            # pair ReduceScatter only. (This branch kept for fallback work.)
            raise NotImplementedError("8-core output fallback not wired yet")

        # ---------- P8: bias + bf16 cast + out ----------
        with tc.tile_pool(name="p8", bufs=2) as wp:
            bo_row = wp.tile([1, DM], F32, name="bo_row")
            nc.sync.dma_start(bo_row[:], WF[2050:2051, :])
            bo_sb = wp.tile([128, DM], F32, name="bo_sb")
            nc.gpsimd.partition_broadcast(bo_sb[:], bo_row[:])
            for tt in range(16):
                xt = wp.tile([128, DM], F32, tag="xt")
                nc.sync.dma_start(xt[:], rsout[tt * 128:(tt + 1) * 128, :])
                ot = wp.tile([128, DM], BF16, tag="ot")
                nc.vector.tensor_add(ot[:], xt[:], bo_sb[:])
                nc.sync.dma_start(y[tt * 128:(tt + 1) * 128, :], ot[:])

        cpool.release()
        drp.release()
    nc.compile()
    return nc


def _get_nc():
    global _NC
    if _NC is None:
        _NC = _build_nc()
    return _NC


def kernel(q, k, v, Wq, bq, Wk, bk, Wv, bv, Wo, bo):
    global LAST_EXEC_NS, LAST_RUN_S
    import time

    import ml_dtypes
    from concourse.bass_utils import run_bass_kernel_spmd

    bf16 = ml_dtypes.bfloat16
    nc = _get_nc()

    q = np.asarray(q, np.float32).reshape(B, 2, RH, DM)
    k = np.asarray(k, np.float32).reshape(B, 2, RH, DM)
    v = np.asarray(v, np.float32).reshape(B, 2, RH, DM).astype(bf16)

    WF = np.zeros((WF_ROWS, DM), np.float32)
    WF[0:1024] = np.asarray(Wq, np.float32).T
    WF[1024:2048] = np.asarray(Wk, np.float32).T
    WF[2048] = np.asarray(bq, np.float32)
    WF[2049] = np.asarray(bk, np.float32)
    WF[2050] = np.asarray(bo, np.float32)
    WF[2051] = np.asarray(bv, np.float32)
    WBl = np.zeros((WB_ROWS, DM), bf16)
    WBl[0:1024] = np.asarray(Wv, np.float32).T.astype(bf16)
    WBl[1024:2048] = np.asarray(Wo, np.float32).T.astype(bf16)
    ident = np.eye(128, dtype=np.float32)

    in_maps = []
    for c in range(NCORES):
        b, g = c // 2, c % 2
        gs = np.zeros((1, 2), np.float32)
        gs[0, g] = 1.0
        in_maps.append({
            "qh": q[b, g], "kh": k[b, g], "vh": v[b, g],
            "wf_sl": WF[c * WSL:(c + 1) * WSL],
            "wb_sl": WBl[c * WSL:(c + 1) * WSL],
            "ident": ident, "gsel": gs,
        })

    t0 = time.time()
    res = run_bass_kernel_spmd(nc, in_maps, core_ids=list(range(NCORES)))
    LAST_RUN_S = time.time() - t0
    LAST_EXEC_NS = res.exec_time_ns

    out = np.concatenate(
        [np.asarray(res.results[c]["y"]).astype(np.float32) for c in range(NCORES)],
        axis=0)
    return out.reshape(B, L, DM)


# revision 3
# speedup vs baseline: 1.0525x; 1.0525x over previous
"""AutoCorrelation block fully on-device (8 NeuronCores).

Core c = 2b+g (b batch, g head-group of 8 heads). Pipeline per core:
  P1  PE-transpose q,k halves (f32)         -> qhT, khT  (DRAM)
  P2  projections Q^T,K^T (f32, +bias), V^T (bf16, +bias)
  P3  pair AllGather -> full-L Q^T,K^T,V^T for the batch
  P5  per head h: M = Q K^T strips (f32 PE), skew-written to DRAM so the
      circular-diagonal sums become plain strided reads; partition-reduce
      -> exact f32 corr_mean; top-3 via max_with_indices; softmax weights;
      V rolled by each delay via runtime-offset DMA from [V^T|V^T]; weighted
      sum -> accV (d-partitioned, bf16)
  P6  out = sum_h accV_h^T @ WoT_h  (bf16 matmul, f32 psum)
  P7  pair ReduceScatter of the partial output
  P8  + bo, cast bf16, write y (2048, 1024)

Numerics: q,k,Wq,Wk f32 end-to-end for delay selection (corr err ~1e-4 vs
min top3-gap 8.4e-3). v/Wv/Wo path bf16.
"""
import os
import sys

import numpy as np

try:
    import concourse.bass  # noqa: F401
except ImportError:
    sys.path.insert(0, "/opt/trn_rl_repo")

B, L, DM = 4, 4096, 1024
H, D, TOPK = 16, 64, 3
NCORES = 8
RH = 2048           # rows per core (half batch)
P = 4224            # Mt row pitch (f32): P*4 % 256 == 0 not required now, P-1 >= 4223
GROUPS2 = [[0, 1], [2, 3], [4, 5], [6, 7]]
GROUPS8 = [[0, 1, 2, 3, 4, 5, 6, 7]]
WF_ROWS = 2056      # f32 blob: WqT(1024) WkT(1024) bq bk bo bv pad
WB_ROWS = 2056      # bf16 blob: WvT(1024) WoT(1024) pad
WSL = WF_ROWS // NCORES  # 257
PAIR_COLL = bool(int(os.environ.get("KV2_PAIR_COLL", "1")))

_NC = None
LAST_EXEC_NS = None
LAST_RUN_S = None


def _build_nc():
    import concourse.bass as bass
    import concourse.mybir as mybir
    import concourse.tile as tile
    from concourse import bacc
    from concourse.ap import AP

    F32, BF16 = mybir.dt.float32, mybir.dt.bfloat16
    U32 = mybir.dt.uint32
    ALU = mybir.AluOpType
    ACT = mybir.ActivationFunctionType
    PS = bass.MemorySpace.PSUM

    nc = bacc.Bacc(None, target_bir_lowering=False, num_devices=NCORES)

    F16 = mybir.dt.float16
    F8 = mybir.dt.float8e4
    qh16 = nc.dram_tensor("qh16", (RH, DM), F16, kind="ExternalInput")
    qr8 = nc.dram_tensor("qr8", (RH, DM), F8, kind="ExternalInput")
    kh16 = nc.dram_tensor("kh16", (RH, DM), F16, kind="ExternalInput")
    kr8 = nc.dram_tensor("kr8", (RH, DM), F8, kind="ExternalInput")
    vh = nc.dram_tensor("vh", (RH, DM), BF16, kind="ExternalInput")
    wf_sl = nc.dram_tensor("wf_sl", (WSL, DM), F32, kind="ExternalInput")
    wb_sl = nc.dram_tensor("wb_sl", (WSL, DM), BF16, kind="ExternalInput")
    ident_in = nc.dram_tensor("ident", (128, 128), F32, kind="ExternalInput")
    gsel = nc.dram_tensor("gsel", (1, 2), F32, kind="ExternalInput")  # one-hot of g
    y = nc.dram_tensor("y", (RH, DM), BF16, kind="ExternalOutput")
    DBG = bool(int(os.environ.get("KV2_DEBUG", "0")))
    if DBG:
        corr_dbg = nc.dram_tensor("corr_dbg", (8, L), F32, kind="ExternalOutput")
        ci_dbg = nc.dram_tensor("ci_dbg", (8, 8), U32, kind="ExternalOutput")
        w3_dbg = nc.dram_tensor("w3_dbg", (8, 8), F32, kind="ExternalOutput")
        qt_dbg = nc.dram_tensor("qt_dbg", (DM, RH), F32, kind="ExternalOutput")
        vt_dbg = nc.dram_tensor("vt_dbg", (RH, DM), BF16, kind="ExternalOutput")
        av_dbg = nc.dram_tensor("av_dbg", (8, D, L), BF16, kind="ExternalOutput")

    NAG = 2 if PAIR_COLL else NCORES   # replicas seen in projection AllGather

    with tile.TileContext(nc) as tc:
        cpool = tc.alloc_tile_pool(name="const", bufs=1)
        drp = tc.alloc_tile_pool(name="dram", bufs=1, space="DRAM")

        ident = cpool.tile([128, 128], F32, name="identt")
        nc.sync.dma_start(ident[:], ident_in[:])
        ones = cpool.tile([128, 1], F32, name="ones")
        nc.vector.memset(ones[:], 1.0)
        gsel_sb = cpool.tile([1, 2], F32, name="gsel_sb")
        nc.sync.dma_start(gsel_sb[:], gsel[:])
        g0b = cpool.tile([128, 1], F32, name="g0b")
        g1b = cpool.tile([128, 1], F32, name="g1b")
        nc.gpsimd.partition_broadcast(g0b[:], gsel_sb[0:1, 0:1])
        nc.gpsimd.partition_broadcast(g1b[:], gsel_sb[0:1, 1:2])

        # ---------- weight AllGather (8-core) ----------
        WF = drp.tile([WF_ROWS, DM], F32, name="WF")
        WB = drp.tile([WB_ROWS, DM], BF16, name="WB")
        wfb = drp.tile([WSL, DM], F32, name="wfb")
        wbb = drp.tile([WSL, DM], BF16, name="wbb")
        nc.sync.dma_start(wfb[:], wf_sl[:])
        nc.sync.dma_start(wbb[:], wb_sl[:])
        nc.gpsimd.collective_compute("AllGather", ALU.bypass, replica_groups=GROUPS8,
                                     ins=[wfb.opt()], outs=[WF.opt()])
        nc.gpsimd.collective_compute("AllGather", ALU.bypass, replica_groups=GROUPS8,
                                     ins=[wbb.opt()], outs=[WB.opt()])

        bq_sb = cpool.tile([128, 8], F32, name="bq_sb")   # [o%128, o//128]
        bk_sb = cpool.tile([128, 8], F32, name="bk_sb")
        for t_sb, row in ((bq_sb, 2048), (bk_sb, 2049)):
            src = AP(tensor=WF[:].tensor, offset=row * DM, ap=[[1, 128], [128, 8]])
            nc.sync.dma_start(t_sb[:], src)
        # row index base for the V-roll gathers: iof[p, j] = 128j + p (f32)
        io32 = cpool.tile([128, 32], mybir.dt.int32, name="io32")
        nc.gpsimd.iota(io32[:], pattern=[[128, 32]], base=0, channel_multiplier=1)
        iof = cpool.tile([128, 32], F32, name="iof")
        nc.vector.tensor_copy(iof[:], io32[:])

        # ---------- DRAM intermediates ----------
        qhT = drp.tile([DM, RH], F32, name="qhT")
        khT = drp.tile([DM, RH], F32, name="khT")
        QTh = drp.tile([DM, RH], F32, name="QTh")
        KTh = drp.tile([DM, RH], F32, name="KTh")
        Vrows = drp.tile([RH, DM], BF16, name="Vrows")
        AGQ = drp.tile([NAG, DM, RH], F32, name="AGQ")
        AGK = drp.tile([NAG, DM, RH], F32, name="AGK")
        AGV = drp.tile([NAG, RH, DM], BF16, name="AGV")
        Vp1 = drp.tile([L, D], BF16, name="Vp1")
        Mt = drp.tile([L, P], F32, name="Mt")
        accV_d = drp.tile([8, D, L], BF16, name="accV_d")
        opart = drp.tile([L, DM], F32, name="opart")
        rsout = drp.tile([RH, DM], F32, name="rsout")

        # ---------- P1: transpose qh, kh (f32, PE) ----------
        with tc.tile_pool(name="p1", bufs=2) as wp, \
             tc.tile_pool(name="p1ps", bufs=4, space=PS) as pp:
            for (s16, s8), dst in (((qh16, qr8), qhT), ((kh16, kr8), khT)):
                for tt in range(16):
                    x16 = wp.tile([128, DM], F16, tag="x16")
                    nc.sync.dma_start(x16[:], s16[tt * 128:(tt + 1) * 128, :])
                    x8 = wp.tile([128, DM], F8, tag="x8")
                    nc.sync.dma_start(x8[:], s8[tt * 128:(tt + 1) * 128, :])
                    xt = wp.tile([128, DM], F32, tag="xt")
                    nc.vector.scalar_tensor_tensor(xt[:], x8[:], 1.0 / 256.0,
                                                   x16[:], op0=ALU.mult,
                                                   op1=ALU.add)
                    for ic in range(8):
                        pt = pp.tile([128, 128], F32, tag="pt")
                        nc.tensor.transpose(pt[:], xt[:, ic * 128:(ic + 1) * 128],
                                            ident[:])
                        st = wp.tile([128, 128], F32, tag="st")
                        nc.vector.tensor_copy(st[:], pt[:])
                        nc.sync.dma_start(
                            dst[ic * 128:(ic + 1) * 128, tt * 128:(tt + 1) * 128],
                            st[:])

        # ---------- P2: projections (all transposed orientation) ----------
        with tc.tile_pool(name="p2w", bufs=1) as wgt, \
             tc.tile_pool(name="p2", bufs=2) as wp, \
             tc.tile_pool(name="p2ps", bufs=4, space=PS) as pp:
            def ld_wT(rows0, blob, dt, name):
                t = wgt.tile([128, 8, DM], dt, name=name)
                src = AP(tensor=blob[:].tensor, offset=rows0 * DM,
                         ap=[[DM, 128], [128 * DM, 8], [1, DM]])
                nc.sync.dma_start(t[:], src)
                return t
            WqT_sb = ld_wT(0, WF, F32, "WqT_sb")
            WkT_sb = ld_wT(1024, WF, F32, "WkT_sb")
            WvT_sb = ld_wT(0, WB, BF16, "WvT_sb")
            vhT = wgt.tile([128, 8, RH], BF16, name="vhT")
            for ic in range(8):
                nc.sync.dma_start_transpose(vhT[:, ic, :],
                                            vh[:, ic * 128:(ic + 1) * 128])

            # f32 Q^T/K^T: lhsT = W chunk, rhs = xT slab (f32)
            for xT, wsb, bsb, dstT in ((qhT, WqT_sb, bq_sb, QTh),
                                       (khT, WkT_sb, bk_sb, KTh)):
                for ts in range(4):
                    xs = wp.tile([128, 8, 512], F32, tag="xs")
                    for ic in range(8):
                        nc.sync.dma_start(
                            xs[:, ic, :],
                            xT[ic * 128:(ic + 1) * 128, ts * 512:(ts + 1) * 512])
                    for ot in range(8):
                        ps = pp.tile([128, 512], F32, tag="ps")
                        for ic in range(8):
                            nc.tensor.matmul(
                                ps[:], wsb[:, ic, ot * 128:(ot + 1) * 128],
                                xs[:, ic, :], start=(ic == 0), stop=(ic == 7))
                        st = wp.tile([128, 512], F32, tag="st")
                        nc.vector.tensor_scalar(st[:], ps[:], bsb[:, ot:ot + 1],
                                                None, op0=ALU.add)
                        nc.sync.dma_start(
                            dstT[ot * 128:(ot + 1) * 128, ts * 512:(ts + 1) * 512],
                            st[:])

            # bf16 V rows: lhsT = vhT chunk (stationary), rhs = WvT chunk
            bv_row = wgt.tile([1, DM], F32, name="bv_row")
            nc.sync.dma_start(bv_row[:], WF[2051:2052, :])
            bvb = wgt.tile([128, DM], F32, name="bvb")
            nc.gpsimd.partition_broadcast(bvb[:], bv_row[:])
            for tt in range(16):
                for osl in range(2):
                    ps = pp.tile([128, 512], F32, tag="ps")
                    for ic in range(8):
                        nc.tensor.matmul(
                            ps[:], vhT[:, ic, tt * 128:(tt + 1) * 128],
                            WvT_sb[:, ic, osl * 512:(osl + 1) * 512],
                            start=(ic == 0), stop=(ic == 7))
                    st = wp.tile([128, 512], BF16, tag="stv")
                    nc.vector.tensor_add(st[:], ps[:],
                                         bvb[:, osl * 512:(osl + 1) * 512])
                    nc.sync.dma_start(
                        Vrows[tt * 128:(tt + 1) * 128, osl * 512:(osl + 1) * 512],
                        st[:])

        if DBG:
            nc.sync.dma_start(qt_dbg[:], QTh[:])
            nc.sync.dma_start(vt_dbg[:], Vrows[:])
        # ---------- P3: AllGather of projections ----------
        pg = GROUPS2 if PAIR_COLL else GROUPS8
        for src_, dst in ((QTh, AGQ), (KTh, AGK), (Vrows, AGV)):
            nc.gpsimd.collective_compute("AllGather", ALU.bypass, replica_groups=pg,
                                         ins=[src_.opt()], outs=[dst.opt()])

        # ---------- P5: per-head ----------
        with tc.tile_pool(name="hd", bufs=1) as hp:
            ACC = hp.tile([128, L], F32, name="ACCt")
            CORR = hp.tile([1, L], F32, name="CORRt")
            accVb = hp.tile([64, L], BF16, name="accVbt")

            for h in range(8):
                with tc.tile_pool(name="qk", bufs=1) as qk:
                    QT_sb = qk.tile([64, L], F32, tag="QT_sb")
                    KT_sb = qk.tile([64, L], F32, tag="KT_sb")
                    tmp = qk.tile([64, L], F32, tag="tmpf")

                    for dstT, ag in ((QT_sb, AGQ), (KT_sb, AGK)):
                        for r in range(2):
                            nc.sync.dma_start(
                                dstT[:, r * RH:(r + 1) * RH],
                                ag[r, 64 * h:64 * h + 64, :])
                            nc.sync.dma_start(
                                tmp[:, r * RH:(r + 1) * RH],
                                ag[r, 512 + 64 * h:512 + 64 * h + 64, :])
                        nc.vector.tensor_scalar(dstT[:], dstT[:], g0b[0:64, :],
                                                None, op0=ALU.mult)
                        nc.vector.scalar_tensor_tensor(
                            dstT[:], tmp[:], g1b[0:64, :], dstT[:],
                            op0=ALU.mult, op1=ALU.add)
                    # stage this head's V rows (g-blended) into Vp1 (L, D)
                    vs0 = qk.tile([128, 32, D], BF16, tag="vs0")
                    vs1 = qk.tile([128, 32, D], BF16, tag="vs1")
                    for r in range(2):
                        for q_, off in ((vs0, 64 * h), (vs1, 512 + 64 * h)):
                            srcv = AP(tensor=AGV[:].tensor,
                                      offset=r * RH * DM + off,
                                      ap=[[DM, 128], [128 * DM, 16], [1, D]])
                            nc.sync.dma_start(q_[:, r * 16:(r + 1) * 16, :], srcv)
                    nc.vector.tensor_scalar(vs0[:], vs0[:], g0b[:], None,
                                            op0=ALU.mult)
                    nc.vector.scalar_tensor_tensor(vs0[:], vs1[:], g1b[:], vs0[:],
                                                   op0=ALU.mult, op1=ALU.add)
                    vp_dst = AP(tensor=Vp1[:].tensor, offset=0,
                                ap=[[D, 128], [128 * D, 32], [1, D]])
                    nc.sync.dma_start(vp_dst, vs0[:])

                    # ---- M strips (f32) -> skewed Mt -> strided reads -> ACC
                    with tc.tile_pool(name="mst", bufs=2) as sp, \
                         tc.tile_pool(name="mps", bufs=2, space=PS) as mp:
                        for J in range(32):
                            strip = sp.tile([128, L], F32, tag="strip")
                            for halfp in range(2):
                                ps = mp.tile([128, 2048], F32, tag="mm")
                                for isl in range(4):
                                    s = halfp * 4 + isl
                                    nc.tensor.matmul(
                                        ps[:, isl * 512:(isl + 1) * 512],
                                        KT_sb[:, J * 128:(J + 1) * 128],
                                        QT_sb[:, s * 512:(s + 1) * 512],
                                        start=True, stop=True,
                                        skip_group_check=True)
                                nc.vector.tensor_copy(
                                    strip[:, halfp * 2048:(halfp + 1) * 2048],
                                    ps[:])
                            main = AP(tensor=Mt[:].tensor, offset=128 * J * P,
                                      ap=[[P - 1, 128], [1, L]])
                            nc.sync.dma_start(main, strip[:])
                            wrap = AP(tensor=Mt[:].tensor, offset=128 * J * P + L,
                                      ap=[[P - 1, 128], [1, 127]])
                            nc.sync.dma_start(wrap, strip[:, 0:127])
                        for J in range(32):
                            sk = sp.tile([128, L], F32, tag="sk")
                            src = AP(tensor=Mt[:].tensor, offset=128 * J * P,
                                     ap=[[P, 128], [1, L]])
                            nc.sync.dma_start(sk[:], src)
                            off = 128 * J
                            if J == 0:
                                nc.vector.tensor_copy(ACC[:], sk[:])
                            else:
                                nc.vector.tensor_add(ACC[:, 0:L - off],
                                                     ACC[:, 0:L - off],
                                                     sk[:, off:L])
                                nc.vector.tensor_add(ACC[:, L - off:L],
                                                     ACC[:, L - off:L],
                                                     sk[:, 0:off])

                    with tc.tile_pool(name="rps", bufs=2, space=PS) as rp:
                        for ns in range(8):
                            pc = rp.tile([1, 512], F32, tag="pc")
                            nc.tensor.matmul(pc[:], ones[:],
                                             ACC[:, ns * 512:(ns + 1) * 512],
                                             start=True, stop=True)
                            nc.vector.tensor_scalar(
                                CORR[0:1, ns * 512:(ns + 1) * 512], pc[:],
                                1.0 / 64, None, op0=ALU.mult)

                    # top-3 delays + softmax weights
                    cv = hp.tile([1, 8], F32, tag="cv")
                    ci = hp.tile([1, 8], U32, tag="ci")
                    nc.vector.max_with_indices(cv[:], ci[:], CORR[:])
                    ex = hp.tile([1, 8], F32, tag="ex")
                    nc.vector.tensor_scalar(ex[:], cv[:], cv[0:1, 0:1], None,
                                            op0=ALU.subtract)
                    nc.scalar.activation(ex[:], ex[:], ACT.Exp)
                    sm = hp.tile([1, 1], F32, tag="sm")
                    nc.vector.tensor_reduce(sm[:], ex[0:1, 0:TOPK],
                                            axis=mybir.AxisListType.X, op=ALU.add)
                    si = hp.tile([1, 1], F32, tag="si")
                    nc.vector.reciprocal(si[:], sm[:])
                    w3 = hp.tile([1, 8], F32, tag="w3")
                    nc.vector.tensor_scalar(w3[:], ex[:], si[0:1, 0:1], None,
                                            op0=ALU.mult)
                    w3b = hp.tile([128, 8], F32, tag="w3b")
                    nc.gpsimd.partition_broadcast(w3b[:], w3[:])

                    # rolled-V weighted sum via indirect row gathers
                    cif = hp.tile([1, 8], F32, tag="cif")
                    nc.vector.tensor_copy(cif[:], ci[:])
                    taub = hp.tile([128, 8], F32, tag="taub")
                    nc.gpsimd.partition_broadcast(taub[:], cif[:])
                    ACCW = hp.tile([128, 32, D], F32, tag="ACCW")
                    for cd in range(TOPK):
                        idxf = hp.tile([128, 32], F32, tag="idxf")
                        nc.vector.tensor_scalar(idxf[:], iof[:],
                                                taub[:, cd:cd + 1], None,
                                                op0=ALU.subtract)
                        mkt = hp.tile([128, 32], F32, tag="mkt")
                        nc.vector.tensor_scalar(mkt[:], idxf[:], 0.0, None,
                                                op0=ALU.is_lt)
                        nc.vector.scalar_tensor_tensor(idxf[:], mkt[:], float(L),
                                                       idxf[:], op0=ALU.mult,
                                                       op1=ALU.add)
                        idx32 = hp.tile([128, 32], mybir.dt.int32, tag="idx32")
                        nc.vector.tensor_copy(idx32[:], idxf[:])
                        vr = qk.tile([128, 32, D], BF16, tag="vr")
                        for J in range(32):
                            nc.gpsimd.indirect_dma_start(
                                out=vr[:, J, :], out_offset=None, in_=Vp1[:],
                                in_offset=bass.IndirectOffsetOnAxis(
                                    ap=idx32[:, J:J + 1], axis=0))
                        if cd == 0:
                            nc.vector.tensor_scalar(ACCW[:], vr[:],
                                                    w3b[:, 0:1], None,
                                                    op0=ALU.mult)
                        else:
                            nc.vector.scalar_tensor_tensor(
                                ACCW[:], vr[:], w3b[:, cd:cd + 1], ACCW[:],
                                op0=ALU.mult, op1=ALU.add)
                    # transpose to (d, t) and store
                    with tc.tile_pool(name="tps", bufs=4, space=PS) as tp2:
                        for T in range(32):
                            pt = tp2.tile([64, 128], F32, tag="pt")
                            nc.tensor.transpose(pt[:], ACCW[:, T, :], ident[:])
                            nc.vector.tensor_copy(
                                accVb[:, T * 128:(T + 1) * 128], pt[:])
                    nc.sync.dma_start(accV_d[h], accVb[:])
                    if DBG:
                        nc.sync.dma_start(corr_dbg[h:h+1, :], CORR[:])
                        nc.sync.dma_start(ci_dbg[h:h+1, :], ci[:])
                        nc.sync.dma_start(w3_dbg[h:h+1, :], w3[:])
                        nc.sync.dma_start(av_dbg[h], accVb[:])

        # ---------- P6: output projection ----------
        with tc.tile_pool(name="p6w", bufs=1) as w6, \
             tc.tile_pool(name="p6", bufs=2) as wp, \
             tc.tile_pool(name="p6ps", bufs=2, space=PS) as pp:
            WoT_all = w6.tile([64, 8, DM], BF16, name="WoT_all")
            wo1 = w6.tile([64, 8, DM], BF16, name="wo1")
            for q_, goff in ((WoT_all, 0), (wo1, 512)):
                src = AP(tensor=WB[:].tensor, offset=(1024 + goff) * DM,
                         ap=[[DM, 64], [64 * DM, 8], [1, DM]])
                nc.sync.dma_start(q_[:], src)
            nc.vector.tensor_scalar(WoT_all[:], WoT_all[:], g0b[0:64, :], None,
                                    op0=ALU.mult)
            nc.vector.scalar_tensor_tensor(WoT_all[:], wo1[:], g1b[0:64, :],
                                           WoT_all[:], op0=ALU.mult, op1=ALU.add)
            for tt in range(32):
                avs = wp.tile([64, 8, 128], BF16, tag="avs")
                for h in range(8):
                    nc.sync.dma_start(avs[:, h, :],
                                      accV_d[h, :, tt * 128:(tt + 1) * 128])
                for ns in range(2):
                    ps = pp.tile([128, 512], F32, tag="ps")
                    for h in range(8):
                        nc.tensor.matmul(ps[:], avs[:, h, :],
                                         WoT_all[:, h, ns * 512:(ns + 1) * 512],
                                         start=(h == 0), stop=(h == 7))
                    st = wp.tile([128, 512], F32, tag="st")
                    nc.vector.tensor_copy(st[:], ps[:])
                    nc.sync.dma_start(
                        opart[tt * 128:(tt + 1) * 128, ns * 512:(ns + 1) * 512],
                        st[:])

        # ---------- P7: pair partial-sum ----------
        if PAIR_COLL:
            nc.gpsimd.collective_compute("ReduceScatter", ALU.add,
                                         replica_groups=GROUPS2,
                                         ins=[opart.opt()], outs=[rsout.opt()])
        else:
            AGO = drp.tile([NCORES, RH, DM], F32, name="AGO")
            half = drp.tile([RH, DM], F32, name="half")
            # each core contributes the rows its PAIR needs? -> allgather my
            # partial's both halves is 2x; instead allgather full partials.
            AGO2 = drp.tile([NCORES, L, DM], F32, name="AGO2")
            nc.gpsimd.collective_compute("AllGather", ALU.bypass,
                                         replica_groups=GROUPS8,
                                         ins=[opart.opt()], outs=[AGO2.opt()])
            # rsout = AGO2[2b][g*RH:...] + AGO2[2b+1][g*RH:...]; rank
            # selection depends on my core id -> use partition-id? Simplest:
            # every core reduces with a gsel/core-id blend is complex; use
            # pair ReduceScatter only. (This branch kept for fallback work.)
            raise NotImplementedError("8-core output fallback not wired yet")

        # ---------- P8: bias + bf16 cast + out ----------
        with tc.tile_pool(name="p8", bufs=2) as wp:
            bo_row = wp.tile([1, DM], F32, name="bo_row")
            nc.sync.dma_start(bo_row[:], WF[2050:2051, :])
            bo_sb = wp.tile([128, DM], F32, name="bo_sb")
            nc.gpsimd.partition_broadcast(bo_sb[:], bo_row[:])
            for tt in range(16):
                xt = wp.tile([128, DM], F32, tag="xt")
                nc.sync.dma_start(xt[:], rsout[tt * 128:(tt + 1) * 128, :])
                ot = wp.tile([128, DM], BF16, tag="ot")
                nc.vector.tensor_add(ot[:], xt[:], bo_sb[:])
                nc.sync.dma_start(y[tt * 128:(tt + 1) * 128, :], ot[:])

        cpool.release()
        drp.release()
    nc.compile()
    return nc


def _get_nc():
    global _NC
    if _NC is None:
        _NC = _build_nc()
    return _NC


def kernel(q, k, v, Wq, bq, Wk, bk, Wv, bv, Wo, bo):
    global LAST_EXEC_NS, LAST_RUN_S
    import time

    import ml_dtypes
    from concourse.bass_utils import run_bass_kernel_spmd

    bf16 = ml_dtypes.bfloat16
    nc = _get_nc()

    f8 = ml_dtypes.float8_e4m3
    q = np.asarray(q, np.float32).reshape(B, 2, RH, DM)
    k = np.asarray(k, np.float32).reshape(B, 2, RH, DM)
    v = np.asarray(v, np.float32).reshape(B, 2, RH, DM).astype(bf16)
    q16 = q.astype(np.float16)
    qr = ((q - q16.astype(np.float32)) * 256.0).astype(f8)
    k16 = k.astype(np.float16)
    kr = ((k - k16.astype(np.float32)) * 256.0).astype(f8)

    WF = np.zeros((WF_ROWS, DM), np.float32)
    WF[0:1024] = np.asarray(Wq, np.float32).T
    WF[1024:2048] = np.asarray(Wk, np.float32).T
    WF[2048] = np.asarray(bq, np.float32)
    WF[2049] = np.asarray(bk, np.float32)
    WF[2050] = np.asarray(bo, np.float32)
    WF[2051] = np.asarray(bv, np.float32)
    WBl = np.zeros((WB_ROWS, DM), bf16)
    WBl[0:1024] = np.asarray(Wv, np.float32).T.astype(bf16)
    WBl[1024:2048] = np.asarray(Wo, np.float32).T.astype(bf16)
    ident = np.eye(128, dtype=np.float32)

    in_maps = []
    for c in range(NCORES):
        b, g = c // 2, c % 2
        gs = np.zeros((1, 2), np.float32)
        gs[0, g] = 1.0
        in_maps.append({
            "qh16": q16[b, g], "qr8": qr[b, g],
            "kh16": k16[b, g], "kr8": kr[b, g], "vh": v[b, g],
            "wf_sl": WF[c * WSL:(c + 1) * WSL],
            "wb_sl": WBl[c * WSL:(c + 1) * WSL],
            "ident": ident, "gsel": gs,
        })

    t0 = time.time()
    res = run_bass_kernel_spmd(nc, in_maps, core_ids=list(range(NCORES)))
    LAST_RUN_S = time.time() - t0
    LAST_EXEC_NS = res.exec_time_ns

    out = np.concatenate(
        [np.asarray(res.results[c]["y"]).astype(np.float32) for c in range(NCORES)],
        axis=0)
    return out.reshape(B, L, DM)


# revision 4
# speedup vs baseline: 1.1173x; 1.0615x over previous
"""AutoCorrelation block fully on-device (8 NeuronCores).

Core c = 2b+g (b batch, g head-group of 8 heads). Pipeline per core:
  P1  PE-transpose q,k halves (f32)         -> qhT, khT  (DRAM)
  P2  projections Q^T,K^T (f32, +bias), V^T (bf16, +bias)
  P3  pair AllGather -> full-L Q^T,K^T,V^T for the batch
  P5  per head h: M = Q K^T strips (f32 PE), skew-written to DRAM so the
      circular-diagonal sums become plain strided reads; partition-reduce
      -> exact f32 corr_mean; top-3 via max_with_indices; softmax weights;
      V rolled by each delay via runtime-offset DMA from [V^T|V^T]; weighted
      sum -> accV (d-partitioned, bf16)
  P6  out = sum_h accV_h^T @ WoT_h  (bf16 matmul, f32 psum)
  P7  pair ReduceScatter of the partial output
  P8  + bo, cast bf16, write y (2048, 1024)

Numerics: q,k,Wq,Wk f32 end-to-end for delay selection (corr err ~1e-4 vs
min top3-gap 8.4e-3). v/Wv/Wo path bf16.
"""
import os
import sys

import numpy as np

try:
    import concourse.bass  # noqa: F401
except ImportError:
    sys.path.insert(0, "/opt/trn_rl_repo")

B, L, DM = 4, 4096, 1024
H, D, TOPK = 16, 64, 3
NCORES = 8
RH = 2048           # rows per core (half batch)
P = 4224            # Mt row pitch (f32): P*4 % 256 == 0 not required now, P-1 >= 4223
GROUPS2 = [[0, 1], [2, 3], [4, 5], [6, 7]]
GROUPS8 = [[0, 1, 2, 3, 4, 5, 6, 7]]
WF_ROWS = 2056      # f32 blob: WqT(1024) WkT(1024) bq bk bo bv pad
WB_ROWS = 2056      # bf16 blob: WvT(1024) WoT(1024) pad
WSL = WF_ROWS // NCORES  # 257
PAIR_COLL = bool(int(os.environ.get("KV2_PAIR_COLL", "1")))

_NC = None
LAST_EXEC_NS = None
LAST_RUN_S = None


def _build_nc():
    import concourse.bass as bass
    import concourse.mybir as mybir
    import concourse.tile as tile
    from concourse import bacc
    from concourse.ap import AP

    F32, BF16 = mybir.dt.float32, mybir.dt.bfloat16
    U32 = mybir.dt.uint32
    ALU = mybir.AluOpType
    ACT = mybir.ActivationFunctionType
    PS = bass.MemorySpace.PSUM

    nc = bacc.Bacc(None, target_bir_lowering=False, num_devices=NCORES)

    F16 = mybir.dt.float16
    F8 = mybir.dt.float8e4
    qh16 = nc.dram_tensor("qh16", (RH, DM), F16, kind="ExternalInput")
    qr8 = nc.dram_tensor("qr8", (RH, DM), F8, kind="ExternalInput")
    kh16 = nc.dram_tensor("kh16", (RH, DM), F16, kind="ExternalInput")
    kr8 = nc.dram_tensor("kr8", (RH, DM), F8, kind="ExternalInput")
    vh = nc.dram_tensor("vh", (RH, DM), BF16, kind="ExternalInput")
    wf_sl = nc.dram_tensor("wf_sl", (WSL, DM), F32, kind="ExternalInput")
    wb_sl = nc.dram_tensor("wb_sl", (WSL, DM), BF16, kind="ExternalInput")
    ident_in = nc.dram_tensor("ident", (128, 128), F32, kind="ExternalInput")
    gsel = nc.dram_tensor("gsel", (1, 2), F32, kind="ExternalInput")  # one-hot of g
    y = nc.dram_tensor("y", (RH, DM), BF16, kind="ExternalOutput")
    DBG = bool(int(os.environ.get("KV2_DEBUG", "0")))
    if DBG:
        corr_dbg = nc.dram_tensor("corr_dbg", (8, L), F32, kind="ExternalOutput")
        ci_dbg = nc.dram_tensor("ci_dbg", (8, 8), U32, kind="ExternalOutput")
        w3_dbg = nc.dram_tensor("w3_dbg", (8, 8), F32, kind="ExternalOutput")
        qt_dbg = nc.dram_tensor("qt_dbg", (DM, RH), F32, kind="ExternalOutput")
        vt_dbg = nc.dram_tensor("vt_dbg", (RH, DM), BF16, kind="ExternalOutput")
        av_dbg = nc.dram_tensor("av_dbg", (8, D, L), BF16, kind="ExternalOutput")

    NAG = 2 if PAIR_COLL else NCORES   # replicas seen in projection AllGather

    with tile.TileContext(nc) as tc:
        cpool = tc.alloc_tile_pool(name="const", bufs=1)
        drp = tc.alloc_tile_pool(name="dram", bufs=1, space="DRAM")

        ident = cpool.tile([128, 128], F32, name="identt")
        nc.sync.dma_start(ident[:], ident_in[:])
        ones = cpool.tile([128, 1], F32, name="ones")
        nc.vector.memset(ones[:], 1.0)
        gsel_sb = cpool.tile([1, 2], F32, name="gsel_sb")
        nc.sync.dma_start(gsel_sb[:], gsel[:])
        g0b = cpool.tile([128, 1], F32, name="g0b")
        g1b = cpool.tile([128, 1], F32, name="g1b")
        nc.gpsimd.partition_broadcast(g0b[:], gsel_sb[0:1, 0:1])
        nc.gpsimd.partition_broadcast(g1b[:], gsel_sb[0:1, 1:2])

        # ---------- weight AllGather (8-core) ----------
        WF = drp.tile([WF_ROWS, DM], F32, name="WF")
        WB = drp.tile([WB_ROWS, DM], BF16, name="WB")
        wfb = drp.tile([WSL, DM], F32, name="wfb")
        wbb = drp.tile([WSL, DM], BF16, name="wbb")
        nc.sync.dma_start(wfb[:], wf_sl[:])
        nc.sync.dma_start(wbb[:], wb_sl[:])
        nc.gpsimd.collective_compute("AllGather", ALU.bypass, replica_groups=GROUPS8,
                                     ins=[wfb.opt()], outs=[WF.opt()])
        nc.gpsimd.collective_compute("AllGather", ALU.bypass, replica_groups=GROUPS8,
                                     ins=[wbb.opt()], outs=[WB.opt()])

        bq_sb = cpool.tile([128, 8], F32, name="bq_sb")   # [o%128, o//128]
        bk_sb = cpool.tile([128, 8], F32, name="bk_sb")
        for t_sb, row in ((bq_sb, 2048), (bk_sb, 2049)):
            src = AP(tensor=WF[:].tensor, offset=row * DM, ap=[[1, 128], [128, 8]])
            nc.sync.dma_start(t_sb[:], src)
        # row index base for the V-roll gathers: iof[p, j] = 128j + p (f32)
        io32 = cpool.tile([128, 32], mybir.dt.int32, name="io32")
        nc.gpsimd.iota(io32[:], pattern=[[128, 32]], base=0, channel_multiplier=1)
        iof = cpool.tile([128, 32], F32, name="iof")
        nc.vector.tensor_copy(iof[:], io32[:])

        # ---------- DRAM intermediates ----------
        qhT = drp.tile([DM, RH], F32, name="qhT")
        khT = drp.tile([DM, RH], F32, name="khT")
        QTh = drp.tile([DM, RH], F32, name="QTh")
        KTh = drp.tile([DM, RH], F32, name="KTh")
        Vrows = drp.tile([RH, DM], BF16, name="Vrows")
        AGQ = drp.tile([NAG, DM, RH], F32, name="AGQ")
        AGK = drp.tile([NAG, DM, RH], F32, name="AGK")
        AGV = drp.tile([NAG, RH, DM], BF16, name="AGV")
        Vp1 = drp.tile([L, D], BF16, name="Vp1")
        Mt = drp.tile([L, P], F32, name="Mt")
        accV_d = drp.tile([8, D, L], BF16, name="accV_d")
        opart = drp.tile([L, DM], F32, name="opart")
        rsout = drp.tile([RH, DM], F32, name="rsout")

        # ---------- P1: transpose qh, kh (f32, PE) ----------
        with tc.tile_pool(name="p1", bufs=2) as wp, \
             tc.tile_pool(name="p1ps", bufs=4, space=PS) as pp:
            for (s16, s8), dst in (((qh16, qr8), qhT), ((kh16, kr8), khT)):
                for tt in range(16):
                    x16 = wp.tile([128, DM], F16, tag="x16")
                    nc.sync.dma_start(x16[:], s16[tt * 128:(tt + 1) * 128, :])
                    x8 = wp.tile([128, DM], F8, tag="x8")
                    nc.sync.dma_start(x8[:], s8[tt * 128:(tt + 1) * 128, :])
                    xt = wp.tile([128, DM], F32, tag="xt")
                    nc.vector.scalar_tensor_tensor(xt[:], x8[:], 1.0 / 256.0,
                                                   x16[:], op0=ALU.mult,
                                                   op1=ALU.add)
                    for ic in range(8):
                        pt = pp.tile([128, 128], F32, tag="pt")
                        nc.tensor.transpose(pt[:], xt[:, ic * 128:(ic + 1) * 128],
                                            ident[:])
                        st = wp.tile([128, 128], F32, tag="st")
                        nc.vector.tensor_copy(st[:], pt[:])
                        nc.sync.dma_start(
                            dst[ic * 128:(ic + 1) * 128, tt * 128:(tt + 1) * 128],
                            st[:])

        # ---------- P2: projections (all transposed orientation) ----------
        with tc.tile_pool(name="p2w", bufs=1) as wgt, \
             tc.tile_pool(name="p2", bufs=2) as wp, \
             tc.tile_pool(name="p2ps", bufs=4, space=PS) as pp:
            def ld_wT(rows0, blob, dt, name):
                t = wgt.tile([128, 8, DM], dt, name=name)
                src = AP(tensor=blob[:].tensor, offset=rows0 * DM,
                         ap=[[DM, 128], [128 * DM, 8], [1, DM]])
                nc.sync.dma_start(t[:], src)
                return t
            WqT_sb = ld_wT(0, WF, F32, "WqT_sb")
            WkT_sb = ld_wT(1024, WF, F32, "WkT_sb")
            WvT_sb = ld_wT(0, WB, BF16, "WvT_sb")
            vhT = wgt.tile([128, 8, RH], BF16, name="vhT")
            for ic in range(8):
                nc.sync.dma_start_transpose(vhT[:, ic, :],
                                            vh[:, ic * 128:(ic + 1) * 128])

            # f32 Q^T/K^T: lhsT = W chunk, rhs = xT slab (f32)
            for xT, wsb, bsb, dstT in ((qhT, WqT_sb, bq_sb, QTh),
                                       (khT, WkT_sb, bk_sb, KTh)):
                for ts in range(4):
                    xs = wp.tile([128, 8, 512], F32, tag="xs")
                    for ic in range(8):
                        nc.sync.dma_start(
                            xs[:, ic, :],
                            xT[ic * 128:(ic + 1) * 128, ts * 512:(ts + 1) * 512])
                    for ot in range(8):
                        ps = pp.tile([128, 512], F32, tag="ps")
                        for ic in range(8):
                            nc.tensor.matmul(
                                ps[:], wsb[:, ic, ot * 128:(ot + 1) * 128],
                                xs[:, ic, :], start=(ic == 0), stop=(ic == 7))
                        st = wp.tile([128, 512], F32, tag="st")
                        nc.vector.tensor_scalar(st[:], ps[:], bsb[:, ot:ot + 1],
                                                None, op0=ALU.add)
                        nc.sync.dma_start(
                            dstT[ot * 128:(ot + 1) * 128, ts * 512:(ts + 1) * 512],
                            st[:])

            # bf16 V rows: lhsT = vhT chunk (stationary), rhs = WvT chunk
            bv_row = wgt.tile([1, DM], F32, name="bv_row")
            nc.sync.dma_start(bv_row[:], WF[2051:2052, :])
            bvb = wgt.tile([128, DM], F32, name="bvb")
            nc.gpsimd.partition_broadcast(bvb[:], bv_row[:])
            for tt in range(16):
                for osl in range(2):
                    ps = pp.tile([128, 512], F32, tag="ps")
                    for ic in range(8):
                        nc.tensor.matmul(
                            ps[:], vhT[:, ic, tt * 128:(tt + 1) * 128],
                            WvT_sb[:, ic, osl * 512:(osl + 1) * 512],
                            start=(ic == 0), stop=(ic == 7))
                    st = wp.tile([128, 512], BF16, tag="stv")
                    nc.vector.tensor_add(st[:], ps[:],
                                         bvb[:, osl * 512:(osl + 1) * 512])
                    nc.sync.dma_start(
                        Vrows[tt * 128:(tt + 1) * 128, osl * 512:(osl + 1) * 512],
                        st[:])

        if DBG:
            nc.sync.dma_start(qt_dbg[:], QTh[:])
            nc.sync.dma_start(vt_dbg[:], Vrows[:])
        # ---------- P3: AllGather of projections ----------
        pg = GROUPS2 if PAIR_COLL else GROUPS8
        for src_, dst in ((QTh, AGQ), (KTh, AGK), (Vrows, AGV)):
            nc.gpsimd.collective_compute("AllGather", ALU.bypass, replica_groups=pg,
                                         ins=[src_.opt()], outs=[dst.opt()])

        # ---------- P5: per-head ----------
        with tc.tile_pool(name="hd", bufs=1) as hp:
            ACC = hp.tile([128, L], F32, name="ACCt")
            CORR = hp.tile([1, L], F32, name="CORRt")
            accVb = hp.tile([64, L], BF16, name="accVbt")

            for h in range(8):
                with tc.tile_pool(name="qk", bufs=1) as qk:
                    QT_sb = qk.tile([64, L], F32, tag="QT_sb")
                    KT_sb = qk.tile([64, L], F32, tag="KT_sb")
                    tmp = qk.tile([64, L], F32, tag="tmpf")

                    for dstT, ag in ((QT_sb, AGQ), (KT_sb, AGK)):
                        for r in range(2):
                            nc.sync.dma_start(
                                dstT[:, r * RH:(r + 1) * RH],
                                ag[r, 64 * h:64 * h + 64, :])
                            nc.sync.dma_start(
                                tmp[:, r * RH:(r + 1) * RH],
                                ag[r, 512 + 64 * h:512 + 64 * h + 64, :])
                        nc.vector.tensor_scalar(dstT[:], dstT[:], g0b[0:64, :],
                                                None, op0=ALU.mult)
                        nc.vector.scalar_tensor_tensor(
                            dstT[:], tmp[:], g1b[0:64, :], dstT[:],
                            op0=ALU.mult, op1=ALU.add)
                    # stage this head's V rows (g-blended) into Vp1 (L, D)
                    vs0 = qk.tile([128, 32, D], BF16, tag="vs0")
                    vs1 = qk.tile([128, 32, D], BF16, tag="vs1")
                    for r in range(2):
                        for q_, off in ((vs0, 64 * h), (vs1, 512 + 64 * h)):
                            srcv = AP(tensor=AGV[:].tensor,
                                      offset=r * RH * DM + off,
                                      ap=[[DM, 128], [128 * DM, 16], [1, D]])
                            nc.sync.dma_start(q_[:, r * 16:(r + 1) * 16, :], srcv)
                    nc.vector.tensor_scalar(vs0[:], vs0[:], g0b[:], None,
                                            op0=ALU.mult)
                    nc.vector.scalar_tensor_tensor(vs0[:], vs1[:], g1b[:], vs0[:],
                                                   op0=ALU.mult, op1=ALU.add)
                    vp_dst = AP(tensor=Vp1[:].tensor, offset=0,
                                ap=[[D, 128], [128 * D, 32], [1, D]])
                    nc.sync.dma_start(vp_dst, vs0[:])

                    # ---- M strips (f32) -> skewed Mt -> strided reads -> ACC
                    with tc.tile_pool(name="mst", bufs=2) as sp, \
                         tc.tile_pool(name="mps", bufs=2, space=PS) as mp:
                        for J in range(32):
                            strip = sp.tile([128, L], F32, tag="strip")
                            for halfp in range(2):
                                ps = mp.tile([128, 2048], F32, tag="mm")
                                for isl in range(4):
                                    s = halfp * 4 + isl
                                    nc.tensor.matmul(
                                        ps[:, isl * 512:(isl + 1) * 512],
                                        KT_sb[:, J * 128:(J + 1) * 128],
                                        QT_sb[:, s * 512:(s + 1) * 512],
                                        start=True, stop=True,
                                        skip_group_check=True)
                                nc.vector.tensor_copy(
                                    strip[:, halfp * 2048:(halfp + 1) * 2048],
                                    ps[:])
                            main = AP(tensor=Mt[:].tensor, offset=128 * J * P,
                                      ap=[[P - 1, 128], [1, L]])
                            nc.sync.dma_start(main, strip[:])
                            wrap = AP(tensor=Mt[:].tensor, offset=128 * J * P + L,
                                      ap=[[P - 1, 128], [1, 127]])
                            nc.sync.dma_start(wrap, strip[:, 0:127])
                        for J in range(32):
                            sk = sp.tile([128, L], F32, tag="sk")
                            src = AP(tensor=Mt[:].tensor, offset=128 * J * P,
                                     ap=[[P, 128], [1, L]])
                            nc.sync.dma_start(sk[:], src)
                            off = 128 * J
                            if J == 0:
                                nc.vector.tensor_copy(ACC[:], sk[:])
                            else:
                                nc.vector.tensor_add(ACC[:, 0:L - off],
                                                     ACC[:, 0:L - off],
                                                     sk[:, off:L])
                                nc.vector.tensor_add(ACC[:, L - off:L],
                                                     ACC[:, L - off:L],
                                                     sk[:, 0:off])

                    with tc.tile_pool(name="rps", bufs=2, space=PS) as rp:
                        for ns in range(8):
                            pc = rp.tile([1, 512], F32, tag="pc")
                            nc.tensor.matmul(pc[:], ones[:],
                                             ACC[:, ns * 512:(ns + 1) * 512],
                                             start=True, stop=True)
                            nc.vector.tensor_scalar(
                                CORR[0:1, ns * 512:(ns + 1) * 512], pc[:],
                                1.0 / 64, None, op0=ALU.mult)

                    # top-3 delays + softmax weights
                    cv = hp.tile([1, 8], F32, tag="cv")
                    ci = hp.tile([1, 8], U32, tag="ci")
                    nc.vector.max_with_indices(cv[:], ci[:], CORR[:])
                    ex = hp.tile([1, 8], F32, tag="ex")
                    nc.vector.tensor_scalar(ex[:], cv[:], cv[0:1, 0:1], None,
                                            op0=ALU.subtract)
                    nc.scalar.activation(ex[:], ex[:], ACT.Exp)
                    sm = hp.tile([1, 1], F32, tag="sm")
                    nc.vector.tensor_reduce(sm[:], ex[0:1, 0:TOPK],
                                            axis=mybir.AxisListType.X, op=ALU.add)
                    si = hp.tile([1, 1], F32, tag="si")
                    nc.vector.reciprocal(si[:], sm[:])
                    w3 = hp.tile([1, 8], F32, tag="w3")
                    nc.vector.tensor_scalar(w3[:], ex[:], si[0:1, 0:1], None,
                                            op0=ALU.mult)
                    w3b = hp.tile([128, 8], F32, tag="w3b")
                    nc.gpsimd.partition_broadcast(w3b[:], w3[:])

                    # rolled-V weighted sum via indirect row gathers
                    cif = hp.tile([1, 8], F32, tag="cif")
                    nc.vector.tensor_copy(cif[:], ci[:])
                    taub = hp.tile([128, 8], F32, tag="taub")
                    nc.gpsimd.partition_broadcast(taub[:], cif[:])
                    ACCW = hp.tile([128, 32, D], F32, tag="ACCW")
                    for cd in range(TOPK):
                        idxf = hp.tile([128, 32], F32, tag="idxf")
                        nc.vector.tensor_scalar(idxf[:], iof[:],
                                                taub[:, cd:cd + 1], None,
                                                op0=ALU.subtract)
                        mkt = hp.tile([128, 32], F32, tag="mkt")
                        nc.vector.tensor_scalar(mkt[:], idxf[:], 0.0, None,
                                                op0=ALU.is_lt)
                        nc.vector.scalar_tensor_tensor(idxf[:], mkt[:], float(L),
                                                       idxf[:], op0=ALU.mult,
                                                       op1=ALU.add)
                        idx32 = hp.tile([128, 32], mybir.dt.int32, tag="idx32")
                        nc.vector.tensor_copy(idx32[:], idxf[:])
                        vr = qk.tile([128, 32, D], BF16, tag="vr")
                        for J in range(32):
                            nc.gpsimd.indirect_dma_start(
                                out=vr[:, J, :], out_offset=None, in_=Vp1[:],
                                in_offset=bass.IndirectOffsetOnAxis(
                                    ap=idx32[:, J:J + 1], axis=0))
                        if cd == 0:
                            nc.vector.tensor_scalar(ACCW[:], vr[:],
                                                    w3b[:, 0:1], None,
                                                    op0=ALU.mult)
                        else:
                            nc.vector.scalar_tensor_tensor(
                                ACCW[:], vr[:], w3b[:, cd:cd + 1], ACCW[:],
                                op0=ALU.mult, op1=ALU.add)
                    # transpose to (d, t) and store
                    with tc.tile_pool(name="tps", bufs=4, space=PS) as tp2:
                        for T in range(32):
                            pt = tp2.tile([64, 128], F32, tag="pt")
                            nc.tensor.transpose(pt[:], ACCW[:, T, :], ident[:])
                            nc.vector.tensor_copy(
                                accVb[:, T * 128:(T + 1) * 128], pt[:])
                    nc.sync.dma_start(accV_d[h], accVb[:])
                    if DBG:
                        nc.sync.dma_start(corr_dbg[h:h+1, :], CORR[:])
                        nc.sync.dma_start(ci_dbg[h:h+1, :], ci[:])
                        nc.sync.dma_start(w3_dbg[h:h+1, :], w3[:])
                        nc.sync.dma_start(av_dbg[h], accVb[:])

        # ---------- P6: output projection ----------
        with tc.tile_pool(name="p6w", bufs=1) as w6, \
             tc.tile_pool(name="p6", bufs=2) as wp, \
             tc.tile_pool(name="p6ps", bufs=2, space=PS) as pp:
            WoT_all = w6.tile([64, 8, DM], BF16, name="WoT_all")
            wo1 = w6.tile([64, 8, DM], BF16, name="wo1")
            for q_, goff in ((WoT_all, 0), (wo1, 512)):
                src = AP(tensor=WB[:].tensor, offset=(1024 + goff) * DM,
                         ap=[[DM, 64], [64 * DM, 8], [1, DM]])
                nc.sync.dma_start(q_[:], src)
            nc.vector.tensor_scalar(WoT_all[:], WoT_all[:], g0b[0:64, :], None,
                                    op0=ALU.mult)
            nc.vector.scalar_tensor_tensor(WoT_all[:], wo1[:], g1b[0:64, :],
                                           WoT_all[:], op0=ALU.mult, op1=ALU.add)
            for tt in range(32):
                avs = wp.tile([64, 8, 128], BF16, tag="avs")
                for h in range(8):
                    nc.sync.dma_start(avs[:, h, :],
                                      accV_d[h, :, tt * 128:(tt + 1) * 128])
                for ns in range(2):
                    ps = pp.tile([128, 512], F32, tag="ps")
                    for h in range(8):
                        nc.tensor.matmul(ps[:], avs[:, h, :],
                                         WoT_all[:, h, ns * 512:(ns + 1) * 512],
                                         start=(h == 0), stop=(h == 7))
                    st = wp.tile([128, 512], F32, tag="st")
                    nc.vector.tensor_copy(st[:], ps[:])
                    nc.sync.dma_start(
                        opart[tt * 128:(tt + 1) * 128, ns * 512:(ns + 1) * 512],
                        st[:])

        # ---------- P7: pair partial-sum ----------
        if PAIR_COLL:
            nc.gpsimd.collective_compute("ReduceScatter", ALU.add,
                                         replica_groups=GROUPS2,
                                         ins=[opart.opt()], outs=[rsout.opt()])
        else:
            AGO = drp.tile([NCORES, RH, DM], F32, name="AGO")
            half = drp.tile([RH, DM], F32, name="half")
            # each core contributes the rows its PAIR needs? -> allgather my
            # partial's both halves is 2x; instead allgather full partials.
            AGO2 = drp.tile([NCORES, L, DM], F32, name="AGO2")
            nc.gpsimd.collective_compute("AllGather", ALU.bypass,
                                         replica_groups=GROUPS8,
                                         ins=[opart.opt()], outs=[AGO2.opt()])
            # rsout = AGO2[2b][g*RH:...] + AGO2[2b+1][g*RH:...]; rank
            # selection depends on my core id -> use partition-id? Simplest:
            # every core reduces with a gsel/core-id blend is complex; use
            # pair ReduceScatter only. (This branch kept for fallback work.)
            raise NotImplementedError("8-core output fallback not wired yet")

        # ---------- P8: bias + bf16 cast + out ----------
        with tc.tile_pool(name="p8", bufs=2) as wp:
            bo_row = wp.tile([1, DM], F32, name="bo_row")
            nc.sync.dma_start(bo_row[:], WF[2050:2051, :])
            bo_sb = wp.tile([128, DM], F32, name="bo_sb")
            nc.gpsimd.partition_broadcast(bo_sb[:], bo_row[:])
            for tt in range(16):
                xt = wp.tile([128, DM], F32, tag="xt")
                nc.sync.dma_start(xt[:], rsout[tt * 128:(tt + 1) * 128, :])
                ot = wp.tile([128, DM], BF16, tag="ot")
                nc.vector.tensor_add(ot[:], xt[:], bo_sb[:])
                nc.sync.dma_start(y[tt * 128:(tt + 1) * 128, :], ot[:])

        cpool.release()
        drp.release()
    nc.compile()
    return nc


def _get_nc():
    global _NC
    if _NC is None:
        _NC = _build_nc()
    return _NC


def kernel(q, k, v, Wq, bq, Wk, bk, Wv, bv, Wo, bo):
    global LAST_EXEC_NS, LAST_RUN_S
    import time

    import ml_dtypes
    from concourse.bass_utils import run_bass_kernel_spmd

    bf16 = ml_dtypes.bfloat16
    nc = _get_nc()

    f8 = ml_dtypes.float8_e4m3
    q = np.asarray(q, np.float32).reshape(B, 2, RH, DM)
    k = np.asarray(k, np.float32).reshape(B, 2, RH, DM)
    v = np.asarray(v, np.float32).reshape(B, 2, RH, DM).astype(bf16)
    q16 = q.astype(np.float16)
    qr = ((q - q16.astype(np.float32)) * 256.0).astype(f8)
    k16 = k.astype(np.float16)
    kr = ((k - k16.astype(np.float32)) * 256.0).astype(f8)

    WF = np.zeros((WF_ROWS, DM), np.float32)
    WF[0:1024] = np.asarray(Wq, np.float32).T
    WF[1024:2048] = np.asarray(Wk, np.float32).T
    WF[2048] = np.asarray(bq, np.float32)
    WF[2049] = np.asarray(bk, np.float32)
    WF[2050] = np.asarray(bo, np.float32)
    WF[2051] = np.asarray(bv, np.float32)
    WBl = np.zeros((WB_ROWS, DM), bf16)
    WBl[0:1024] = np.asarray(Wv, np.float32).T.astype(bf16)
    WBl[1024:2048] = np.asarray(Wo, np.float32).T.astype(bf16)
    ident = np.eye(128, dtype=np.float32)

    in_maps = []
    for c in range(NCORES):
        b, g = c // 2, c % 2
        gs = np.zeros((1, 2), np.float32)
        gs[0, g] = 1.0
        in_maps.append({
            "qh16": q16[b, g], "qr8": qr[b, g],
            "kh16": k16[b, g], "kr8": kr[b, g], "vh": v[b, g],
            "wf_sl": WF[c * WSL:(c + 1) * WSL],
            "wb_sl": WBl[c * WSL:(c + 1) * WSL],
            "ident": ident, "gsel": gs,
        })

    trace = bool(int(os.environ.get("KERNEL_TRACE", "0")))
    t0 = time.time()
    res = run_bass_kernel_spmd(nc, in_maps, core_ids=list(range(NCORES)),
                               trace=trace)
    LAST_RUN_S = time.time() - t0
    LAST_EXEC_NS = res.exec_time_ns

    out = np.concatenate(
        [np.asarray(res.results[c]["y"]).astype(np.float32) for c in range(NCORES)],
        axis=0)
    return out.reshape(B, L, DM)
